# revision 19
# baseline (speedup 1.0000x reference)
"""Trainium2 Bass kernel for windowed mean-pooling (segment_reduce).

Computes, for each (batch b, window w):
    out[b, w, :] = mean over t in [begins[b,w], ends'[b,w]) of features[b, t, :]
where ends' = clip(ends, begins, begins + 8) (the reference gathers at most
MAX_WINDOW=8 tokens) and empty windows produce 0 (count clamped to >= 1).

Strategy (data-parallel over batch, one sample per NeuronCore). The kernel is
bound by DMA queue time (each descriptor line costs ~87 ns + bytes/43 GB/s on
one of 16 queues) with the TensorEngine a close second, so the design
minimizes both:
  - TOKEN COMPACTION: for each block of 128 consecutive (sorted) windows,
    the host packs just the distinct tokens that block touches (~230) into
    ceil(n/128) K-tiles -> 39 K-tile matmuls total instead of 62 dense ones
    (PE: ~13 us instead of ~20 us). The K-tile count per block is the max
    over the 8 cores so one SPMD program serves all (padded slots carry
    zero features and an out-of-range token id -> zero mask).
  - features fp16 [P, NTILES, D] in chunks of 2-8 K-tiles -> 3-12 KB
    per-partition DMA lines, split across both HWDGE rings in K order.
  - masks: tiles for the first HBLK blocks ship as a small host-built fp8
    head (the PE consumes fp8 stationary directly vs fp16 moving), so the
    first matmuls skip the on-device broadcast/compare chain; remaining
    tiles are built on DVE from an 8 KB begins/ends row broadcast across
    partitions by K=1 ones-matmuls ((b<=t)*(t<e) vs the shipped token-id
    column of each compacted K-tile).
  - out_block = S^T @ F accumulated in PSUM (512+256 col split), scaled by
    1/count on ACT, written fp16 to [P, NBLK, D] DRAM in groups (8,4,2,1,1)
    (12 KB lines early, small at the latency-critical tail); the host
    un-shuffles and upcasts.
  - warm-up matmuls at t=0 ramp the PE p-state (0.65 -> 2.4 GHz needs ~3 us
    of continuous execution) while the DMA rings spin up (~3 us).
"""

import os
import sys

import numpy as np

for _p in ("/opt/trn_rl_repo", "/root/.axon_site/_ro/trn_rl_repo"):
    if os.path.isdir(_p) and _p not in sys.path:
        sys.path.insert(0, _p)

import ml_dtypes  # noqa: E402

from concourse import bacc, mybir  # noqa: E402
import concourse.tile as tile  # noqa: E402
from concourse.bass_utils import run_bass_kernel_spmd  # noqa: E402

B, T, D, W = 8, 4096, 768, 2048
MAXWIN = 8
P = 128
NBLK = W // P  # 16 window blocks of 128 windows
HBLK = 6  # leading blocks whose masks ship as the host fp8 head
NWARM = 7  # PE warm-up matmuls ([P, 512] each)
MCH = 512  # windows per metadata broadcast matmul
F32 = mybir.dt.float32
FP16 = mybir.dt.float16
FP8 = mybir.dt.float8e4

FP8NP = ml_dtypes.float8_e4m3


def _fchunks(ntiles):
    """Feature DMA chunk sizes (K-tiles): small first for an early PE start,
    12 KB lines later."""
    sizes = [2, 2, 4, 8]
    left = ntiles - sum(sizes)
    while left > 8:
        sizes.append(8)
        left -= 8
    if left:
        sizes.append(left)
    return sizes


def _build_program(kpb):
    """Build the SPMD Bass program given K-tiles-per-block kpb[NBLK]."""
    nc = bacc.Bacc(None)
    k0s = np.concatenate([[0], np.cumsum(kpb)]).astype(int)  # block k offsets
    ntiles = int(k0s[-1])
    assert ntiles + NBLK <= 64, ntiles  # tokid + iv fit the [P, 64] ioiv
    hw8 = int(k0s[HBLK]) * P  # host mask head columns

    fhi_d = nc.declare_dram_parameter("fhi", [P, ntiles, D], FP16, isOutput=False)
    m8_d = nc.declare_dram_parameter("mask8h", [P, hw8], FP8, isOutput=False)
    meta_d = nc.declare_dram_parameter("meta", [1, 2, W], FP16, isOutput=False)
    ioiv_d = nc.declare_dram_parameter("ioiv", [P, 64], F32, isOutput=False)
    out_d = nc.declare_dram_parameter("out", [P, NBLK, D], FP16, isOutput=True)

    fhi_r = fhi_d[:]
    out_r = out_d[:]

    with tile.TileContext(nc) as tc:
        with (
            tc.tile_pool(name="warmp", bufs=1) as warm_pool,
            tc.tile_pool(name="metap", bufs=1) as meta_pool,
            tc.tile_pool(name="fslab", bufs=1) as f_pool,
            tc.tile_pool(name="m2p", bufs=4) as m2_pool,
            tc.tile_pool(name="maskp", bufs=12) as mask_pool,
            tc.tile_pool(name="outp", bufs=3) as out_pool,
            tc.tile_pool(name="psum", bufs=4, space="PSUM") as psum_pool,
        ):
            # --- PE warm-up: ramp the tensor engine p-state while DMAs start.
            warm_sb = warm_pool.tile([P, 512], FP16)
            nc.vector.memset(warm_sb[:], 0.0)
            for j in range(NWARM):
                wp = psum_pool.tile([P, 512], F32, name=f"warm{j}", tag="ps")
                nc.tensor.matmul(
                    wp[:], warm_sb[:, 0:P], warm_sb[:], start=True, stop=True
                )

            # --- metadata DMAs (small, first on the SP ring).
            m8_sb = meta_pool.tile([P, hw8], FP8)
            nc.sync.dma_start(out=m8_sb[:], in_=m8_d[:])
            meta_sb = meta_pool.tile([1, 2, W], FP16)
            nc.sync.dma_start(out=meta_sb[:], in_=meta_d[:])
            ioiv_sb = meta_pool.tile([P, 64], F32)
            nc.sync.dma_start(out=ioiv_sb[:], in_=ioiv_d[:])
            io_sb = ioiv_sb[:, 0:ntiles]
            iv_sb = ioiv_sb[:, ntiles : ntiles + NBLK]

            # --- feature slab chunks (fp16), ALL on the ACT ring: the DMA
            # queues drain lines in enqueue order, so a single ring keeps
            # K-tile arrival in consumption order (two rings interleave by
            # dispatch time and let late-K chunks jump early-K ones).
            fhi_tiles = []
            k2chunk = []
            k0 = 0
            for j, sz in enumerate(_fchunks(ntiles)):
                fh = f_pool.tile([P, sz, D], FP16, name=f"fh{j}", tag=f"fh{j}")
                eng = nc.scalar
                eng.dma_start(out=fh[:], in_=fhi_r[:, k0 : k0 + sz, :])
                fhi_tiles.append(fh)
                for s in range(sz):
                    k2chunk.append((j, s))
                k0 += sz
            assert k0 == ntiles

            # --- broadcast begins/ends across partitions (only the window
            # chunks that device-built mask tiles consume).
            smin = HBLK * P // MCH
            ones_sb = meta_pool.tile([1, P], FP16)
            nc.vector.memset(ones_sb[:], 1.0)
            be_sb = meta_pool.tile([P, 2, W], FP16)
            for s in range(smin, W // MCH):
                for h in range(2):
                    sl = slice(s * MCH, (s + 1) * MCH)
                    pb = psum_pool.tile([P, MCH], F32, name=f"pb{h}_{s}", tag="ps")
                    nc.tensor.matmul(
                        pb[:], ones_sb[:], meta_sb[:, h, sl], start=True, stop=True
                    )
                    nc.vector.tensor_copy(out=be_sb[:, h, sl], in_=pb[:])

            # --- mask tiles for blocks >= HBLK on DVE:
            # mask[p, w] = (b[w] <= tokid) * (tokid < e[w]).
            dmasks = {}
            for i in range(HBLK, NBLK):
                wlo, whi = i * P, (i + 1) * P
                for k in range(kpb[i]):
                    kc = int(k0s[i]) + k
                    m2 = m2_pool.tile([P, P], FP16, name=f"m2_{kc}", tag="m2")
                    msk = mask_pool.tile([P, P], FP16, name=f"mask_{kc}", tag="mask")
                    nc.vector.tensor_scalar(
                        m2[:], be_sb[:, 1, wlo:whi], io_sb[:, kc : kc + 1], None,
                        mybir.AluOpType.is_gt,
                    )
                    nc.vector.scalar_tensor_tensor(
                        msk[:], be_sb[:, 0, wlo:whi], io_sb[:, kc : kc + 1], m2[:],
                        mybir.AluOpType.is_le, mybir.AluOpType.mult,
                    )
                    dmasks[kc] = msk

            # --- block matmuls + ACT evacuation + grouped output DMA.
            ogroups = (8, 4, 4)
            og_starts = []
            o0 = 0
            for g in ogroups:
                og_starts.append(o0)
                o0 += g
            assert o0 == NBLK

            gi = 0
            os_tile = None
            for i in range(NBLK):
                if i == og_starts[gi]:
                    os_tile = out_pool.tile(
                        [P, ogroups[gi], D], FP16, name=f"os{gi}", tag="os"
                    )
                ps = psum_pool.tile([P, D], F32, name=f"ps{i}", tag="ps")
                for k in range(kpb[i]):
                    kc = int(k0s[i]) + k
                    if i < HBLK:
                        lh = m8_sb[:, kc * P : (kc + 1) * P]
                    else:
                        lh = dmasks[kc]
                    cj, cs = k2chunk[kc]
                    rh = fhi_tiles[cj][:, cs, :]
                    first = k == 0
                    last = k == kpb[i] - 1
                    for n0, nn in ((0, 512), (512, 256)):
                        nc.tensor.matmul(
                            ps[:, n0 : n0 + nn], lh, rh[:, n0 : n0 + nn],
                            start=first, stop=(last and n0 == 512),
                        )
                if i % 2 == 0:
                    nc.scalar.mul(
                        out=os_tile[:, i - og_starts[gi], :], in_=ps[:],
                        mul=iv_sb[:, i : i + 1],
                    )
                else:
                    nc.vector.tensor_scalar(
                        os_tile[:, i - og_starts[gi], :], ps[:],
                        iv_sb[:, i : i + 1], None, mybir.AluOpType.mult,
                    )
                if i == og_starts[gi] + ogroups[gi] - 1:
                    # outputs on the SAME ring as the features: rings hand
                    # lines to the shared queues as they drain, so a second
                    # ring's output lines would interleave with (and delay)
                    # the feature tail; one ring gives strict FIFO order.
                    nc.scalar.dma_start(
                        out=out_r[:, og_starts[gi] : i + 1, :], in_=os_tile[:]
                    )
                    gi += 1

    nc.finalize()
    return nc


def _prepare(features, begins, ends):
    feats = np.asarray(features, dtype=np.float32)
    assert feats.shape == (B, T, D), feats.shape
    b = np.clip(np.asarray(begins).astype(np.int64), 0, T - 1)
    e = np.asarray(ends).astype(np.int64)
    # Reference gathers at most MAXWIN tokens starting at b; empty -> count 1.
    e_eff = np.clip(e, b, np.minimum(b + MAXWIN, T))
    counts = np.maximum(e_eff - b, 1).astype(np.float32)
    inv = (1.0 / counts).astype(np.float32)

    # distinct tokens per (core, block); K-tiles per block = max over cores.
    toks = {}
    kpb = np.zeros(NBLK, int)
    for c in range(B):
        for i in range(NBLK):
            ws = slice(i * P, (i + 1) * P)
            m = np.zeros(T, bool)
            for bb, ee in zip(b[c, ws], e_eff[c, ws]):
                m[bb:ee] = True
            u = np.flatnonzero(m)
            toks[(c, i)] = u
            kpb[i] = max(kpb[i], (len(u) + P - 1) // P)
    k0s = np.concatenate([[0], np.cumsum(kpb)]).astype(int)
    ntiles = int(k0s[-1])
    hw8 = int(k0s[HBLK]) * P

    f16 = feats.astype(np.float16)
    fhi = np.zeros((B, P, ntiles, D), np.float16)
    tokid = np.full((B, P, ntiles), -3000.0, np.float32)  # pad -> mask 0
    mask8 = np.zeros((B, P, hw8), dtype=FP8NP)
    for c in range(B):
        for i in range(NBLK):
            u = toks[(c, i)]
            n = len(u)
            nk = (n + P - 1) // P
            base = int(k0s[i])
            for k in range(nk):
                seg = u[k * P : (k + 1) * P]
                fhi[c, : len(seg), base + k, :] = f16[c, seg, :]
                tokid[c, : len(seg), base + k] = seg - 2048
            if i < HBLK:
                # host fp8 mask tiles for this block (0/1 exact in fp8)
                wlo = i * P
                t_col = tokid[c, :, base : base + kpb[i]]  # [P, kpb]
                bb = b[c, wlo : wlo + P] - 2048
                ee = e_eff[c, wlo : wlo + P] - 2048
                m = (bb[None, None, :] <= t_col[:, :, None]) & (
                    t_col[:, :, None] < ee[None, None, :]
                )  # [P, kpb, 128w]
                for k in range(kpb[i]):
                    mask8[c, :, (base + k) * P : (base + k + 1) * P] = m[
                        :, k, :
                    ].astype(FP8NP)

    in_maps = []
    for c in range(B):
        metac = np.ascontiguousarray(
            (np.stack([b[c], e_eff[c]]) - 2048).astype(np.float16).reshape(1, 2, W)
        )
        ioiv = np.zeros((P, 64), np.float32)
        ioiv[:, 0:ntiles] = tokid[c]
        ioiv[:, ntiles : ntiles + NBLK] = inv[c].reshape(NBLK, P).T
        in_maps.append(
            {"fhi": fhi[c], "mask8h": mask8[c], "meta": metac, "ioiv": ioiv}
        )
    return list(kpb), in_maps


def run(features, begins, ends, trace=False):
    """Build + run on 8 NeuronCores; returns (output, BassKernelResults)."""
    kpb, in_maps = _prepare(features, begins, ends)
    nc = _build_program(kpb)
    res = run_bass_kernel_spmd(nc, in_maps, list(range(B)), trace=trace)
    # out is [P, NBLK, D] fp16 with window w = i*128 + p at [p, i, :]
    out = np.stack(
        [
            np.ascontiguousarray(
                res.results[c]["out"].astype(np.float32).transpose(1, 0, 2)
            ).reshape(W, D)
            for c in range(B)
        ],
        axis=0,
    )
    return out, res


def kernel(features, begins, ends):
    out, _ = run(features, begins, ends, trace=False)
    return out


# revision 20
# speedup vs baseline: 1.0102x; 1.0102x over previous
"""Trainium2 Bass kernel for windowed mean-pooling (segment_reduce).

Computes, for each (batch b, window w):
    out[b, w, :] = mean over t in [begins[b,w], ends'[b,w]) of features[b, t, :]
where ends' = clip(ends, begins, begins + 8) (the reference gathers at most
MAX_WINDOW=8 tokens) and empty windows produce 0 (count clamped to >= 1).

Strategy (data-parallel over batch, one sample per NeuronCore). The kernel is
bound by DMA queue time (each descriptor line costs ~87 ns + bytes/43 GB/s on
one of 16 queues) with the TensorEngine a close second, so the design
minimizes both:
  - TOKEN COMPACTION: for each block of 128 consecutive (sorted) windows,
    the host packs just the distinct tokens that block touches (~230) into
    ceil(n/128) K-tiles -> 39 K-tile matmuls total instead of 62 dense ones
    (PE: ~13 us instead of ~20 us). The K-tile count per block is the max
    over the 8 cores so one SPMD program serves all (padded slots carry
    zero features and an out-of-range token id -> zero mask).
  - features fp16 [P, NTILES, D] in chunks of 2-8 K-tiles -> 3-12 KB
    per-partition DMA lines, split across both HWDGE rings in K order.
  - masks: tiles for the first HBLK blocks ship as a small host-built fp8
    head (the PE consumes fp8 stationary directly vs fp16 moving), so the
    first matmuls skip the on-device broadcast/compare chain; remaining
    tiles are built on DVE from an 8 KB begins/ends row broadcast across
    partitions by K=1 ones-matmuls ((b<=t)*(t<e) vs the shipped token-id
    column of each compacted K-tile).
  - out_block = S^T @ F accumulated in PSUM (512+256 col split), scaled by
    1/count on ACT, written fp16 to [P, NBLK, D] DRAM in groups (8,4,2,1,1)
    (12 KB lines early, small at the latency-critical tail); the host
    un-shuffles and upcasts.
  - warm-up matmuls at t=0 ramp the PE p-state (0.65 -> 2.4 GHz needs ~3 us
    of continuous execution) while the DMA rings spin up (~3 us).
"""

import os
import sys

import numpy as np

for _p in ("/opt/trn_rl_repo", "/root/.axon_site/_ro/trn_rl_repo"):
    if os.path.isdir(_p) and _p not in sys.path:
        sys.path.insert(0, _p)

import ml_dtypes  # noqa: E402

from concourse import bacc, mybir  # noqa: E402
import concourse.tile as tile  # noqa: E402
from concourse.bass_utils import run_bass_kernel_spmd  # noqa: E402

B, T, D, W = 8, 4096, 768, 2048
MAXWIN = 8
P = 128
NBLK = W // P  # 16 window blocks of 128 windows
HBLK = 6  # leading blocks whose masks ship as the host fp8 head
NWARM = 7  # PE warm-up matmuls ([P, 512] each)
MCH = 512  # windows per metadata broadcast matmul
F32 = mybir.dt.float32
FP16 = mybir.dt.float16
FP8 = mybir.dt.float8e4

FP8NP = ml_dtypes.float8_e4m3


def _fchunks(ntiles):
    """Feature DMA chunk sizes (K-tiles): small first for an early PE start,
    12 KB lines later."""
    sizes = [2, 2, 4, 8]
    left = ntiles - sum(sizes)
    while left > 8:
        sizes.append(8)
        left -= 8
    if left:
        sizes.append(left)
    return sizes


def _build_program(kpb):
    """Build the SPMD Bass program given K-tiles-per-block kpb[NBLK]."""
    nc = bacc.Bacc(None)
    k0s = np.concatenate([[0], np.cumsum(kpb)]).astype(int)  # block k offsets
    ntiles = int(k0s[-1])
    assert ntiles + NBLK <= 64, ntiles  # tokid + iv fit the [P, 64] ioiv
    hw8 = int(k0s[HBLK]) * P  # host mask head columns

    fhi_d = nc.declare_dram_parameter("fhi", [P, ntiles, D], FP16, isOutput=False)
    m8_d = nc.declare_dram_parameter("mask8h", [P, hw8], FP8, isOutput=False)
    meta_d = nc.declare_dram_parameter("meta", [1, 2, W], FP16, isOutput=False)
    ioiv_d = nc.declare_dram_parameter("ioiv", [P, 64], F32, isOutput=False)
    out_d = nc.declare_dram_parameter("out", [P, NBLK, D], FP16, isOutput=True)

    fhi_r = fhi_d[:]
    out_r = out_d[:]

    with tile.TileContext(nc) as tc:
        with (
            tc.tile_pool(name="warmp", bufs=1) as warm_pool,
            tc.tile_pool(name="metap", bufs=1) as meta_pool,
            tc.tile_pool(name="fslab", bufs=1) as f_pool,
            tc.tile_pool(name="m2p", bufs=4) as m2_pool,
            tc.tile_pool(name="maskp", bufs=12) as mask_pool,
            tc.tile_pool(name="outp", bufs=2) as out_pool,
            tc.tile_pool(name="psum", bufs=4, space="PSUM") as psum_pool,
        ):
            # --- PE warm-up: ramp the tensor engine p-state while DMAs start.
            warm_sb = warm_pool.tile([P, 512], FP16)
            nc.vector.memset(warm_sb[:], 0.0)
            for j in range(NWARM):
                wp = psum_pool.tile([P, 512], F32, name=f"warm{j}", tag="ps")
                nc.tensor.matmul(
                    wp[:], warm_sb[:, 0:P], warm_sb[:], start=True, stop=True
                )

            # --- metadata DMAs (small, first on the SP ring).
            m8_sb = meta_pool.tile([P, hw8], FP8)
            nc.sync.dma_start(out=m8_sb[:], in_=m8_d[:])
            meta_sb = meta_pool.tile([1, 2, W], FP16)
            nc.sync.dma_start(out=meta_sb[:], in_=meta_d[:])
            ioiv_sb = meta_pool.tile([P, 64], F32)
            nc.sync.dma_start(out=ioiv_sb[:], in_=ioiv_d[:])
            io_sb = ioiv_sb[:, 0:ntiles]
            iv_sb = ioiv_sb[:, ntiles : ntiles + NBLK]

            # --- feature slab chunks (fp16), ALL on the ACT ring: the DMA
            # queues drain lines in enqueue order, so a single ring keeps
            # K-tile arrival in consumption order (two rings interleave by
            # dispatch time and let late-K chunks jump early-K ones).
            fhi_tiles = []
            k2chunk = []
            k0 = 0
            for j, sz in enumerate(_fchunks(ntiles)):
                fh = f_pool.tile([P, sz, D], FP16, name=f"fh{j}", tag=f"fh{j}")
                eng = nc.scalar
                eng.dma_start(out=fh[:], in_=fhi_r[:, k0 : k0 + sz, :])
                fhi_tiles.append(fh)
                for s in range(sz):
                    k2chunk.append((j, s))
                k0 += sz
            assert k0 == ntiles

            # --- broadcast begins/ends across partitions (only the window
            # chunks that device-built mask tiles consume).
            smin = HBLK * P // MCH
            ones_sb = meta_pool.tile([1, P], FP16)
            nc.vector.memset(ones_sb[:], 1.0)
            be_sb = meta_pool.tile([P, 2, W], FP16)
            for s in range(smin, W // MCH):
                for h in range(2):
                    sl = slice(s * MCH, (s + 1) * MCH)
                    pb = psum_pool.tile([P, MCH], F32, name=f"pb{h}_{s}", tag="ps")
                    nc.tensor.matmul(
                        pb[:], ones_sb[:], meta_sb[:, h, sl], start=True, stop=True
                    )
                    nc.vector.tensor_copy(out=be_sb[:, h, sl], in_=pb[:])

            # --- mask tiles for blocks >= HBLK on DVE:
            # mask[p, w] = (b[w] <= tokid) * (tokid < e[w]).
            dmasks = {}
            for i in range(HBLK, NBLK):
                wlo, whi = i * P, (i + 1) * P
                for k in range(kpb[i]):
                    kc = int(k0s[i]) + k
                    m2 = m2_pool.tile([P, P], FP16, name=f"m2_{kc}", tag="m2")
                    msk = mask_pool.tile([P, P], FP16, name=f"mask_{kc}", tag="mask")
                    nc.vector.tensor_scalar(
                        m2[:], be_sb[:, 1, wlo:whi], io_sb[:, kc : kc + 1], None,
                        mybir.AluOpType.is_gt,
                    )
                    nc.vector.scalar_tensor_tensor(
                        msk[:], be_sb[:, 0, wlo:whi], io_sb[:, kc : kc + 1], m2[:],
                        mybir.AluOpType.is_le, mybir.AluOpType.mult,
                    )
                    dmasks[kc] = msk

            # --- block matmuls + ACT evacuation + grouped output DMA.
            ogroups = (12, 4)
            og_starts = []
            o0 = 0
            for g in ogroups:
                og_starts.append(o0)
                o0 += g
            assert o0 == NBLK

            gi = 0
            os_tile = None
            for i in range(NBLK):
                if i == og_starts[gi]:
                    os_tile = out_pool.tile(
                        [P, ogroups[gi], D], FP16, name=f"os{gi}", tag="os"
                    )
                ps = psum_pool.tile([P, D], F32, name=f"ps{i}", tag="ps")
                for k in range(kpb[i]):
                    kc = int(k0s[i]) + k
                    if i < HBLK:
                        lh = m8_sb[:, kc * P : (kc + 1) * P]
                    else:
                        lh = dmasks[kc]
                    cj, cs = k2chunk[kc]
                    rh = fhi_tiles[cj][:, cs, :]
                    first = k == 0
                    last = k == kpb[i] - 1
                    for n0, nn in ((0, 512), (512, 256)):
                        nc.tensor.matmul(
                            ps[:, n0 : n0 + nn], lh, rh[:, n0 : n0 + nn],
                            start=first, stop=(last and n0 == 512),
                        )
                if i % 2 == 0:
                    nc.scalar.mul(
                        out=os_tile[:, i - og_starts[gi], :], in_=ps[:],
                        mul=iv_sb[:, i : i + 1],
                    )
                else:
                    nc.vector.tensor_scalar(
                        os_tile[:, i - og_starts[gi], :], ps[:],
                        iv_sb[:, i : i + 1], None, mybir.AluOpType.mult,
                    )
                if i == og_starts[gi] + ogroups[gi] - 1:
                    # outputs on the SP ring; the first group spans 12
                    # blocks so its DMA only fires after the feature stream
                    # has drained (rings hand lines to the shared queues as
                    # they drain, so an early output DMA on the other ring
                    # would interleave with and delay the feature tail).
                    nc.sync.dma_start(
                        out=out_r[:, og_starts[gi] : i + 1, :], in_=os_tile[:]
                    )
                    gi += 1

    nc.finalize()
    return nc


def _prepare(features, begins, ends):
    feats = np.asarray(features, dtype=np.float32)
    assert feats.shape == (B, T, D), feats.shape
    b = np.clip(np.asarray(begins).astype(np.int64), 0, T - 1)
    e = np.asarray(ends).astype(np.int64)
    # Reference gathers at most MAXWIN tokens starting at b; empty -> count 1.
    e_eff = np.clip(e, b, np.minimum(b + MAXWIN, T))
    counts = np.maximum(e_eff - b, 1).astype(np.float32)
    inv = (1.0 / counts).astype(np.float32)

    # distinct tokens per (core, block); K-tiles per block = max over cores.
    toks = {}
    kpb = np.zeros(NBLK, int)
    for c in range(B):
        for i in range(NBLK):
            ws = slice(i * P, (i + 1) * P)
            m = np.zeros(T, bool)
            for bb, ee in zip(b[c, ws], e_eff[c, ws]):
                m[bb:ee] = True
            u = np.flatnonzero(m)
            toks[(c, i)] = u
            kpb[i] = max(kpb[i], (len(u) + P - 1) // P)
    k0s = np.concatenate([[0], np.cumsum(kpb)]).astype(int)
    ntiles = int(k0s[-1])
    hw8 = int(k0s[HBLK]) * P

    f16 = feats.astype(np.float16)
    fhi = np.zeros((B, P, ntiles, D), np.float16)
    tokid = np.full((B, P, ntiles), -3000.0, np.float32)  # pad -> mask 0
    mask8 = np.zeros((B, P, hw8), dtype=FP8NP)
    for c in range(B):
        for i in range(NBLK):
            u = toks[(c, i)]
            n = len(u)
            nk = (n + P - 1) // P
            base = int(k0s[i])
            for k in range(nk):
                seg = u[k * P : (k + 1) * P]
                fhi[c, : len(seg), base + k, :] = f16[c, seg, :]
                tokid[c, : len(seg), base + k] = seg - 2048
            if i < HBLK:
                # host fp8 mask tiles for this block (0/1 exact in fp8)
                wlo = i * P
                t_col = tokid[c, :, base : base + kpb[i]]  # [P, kpb]
                bb = b[c, wlo : wlo + P] - 2048
                ee = e_eff[c, wlo : wlo + P] - 2048
                m = (bb[None, None, :] <= t_col[:, :, None]) & (
                    t_col[:, :, None] < ee[None, None, :]
                )  # [P, kpb, 128w]
                for k in range(kpb[i]):
                    mask8[c, :, (base + k) * P : (base + k + 1) * P] = m[
                        :, k, :
                    ].astype(FP8NP)

    in_maps = []
    for c in range(B):
        metac = np.ascontiguousarray(
            (np.stack([b[c], e_eff[c]]) - 2048).astype(np.float16).reshape(1, 2, W)
        )
        ioiv = np.zeros((P, 64), np.float32)
        ioiv[:, 0:ntiles] = tokid[c]
        ioiv[:, ntiles : ntiles + NBLK] = inv[c].reshape(NBLK, P).T
        in_maps.append(
            {"fhi": fhi[c], "mask8h": mask8[c], "meta": metac, "ioiv": ioiv}
        )
    return list(kpb), in_maps


def run(features, begins, ends, trace=False):
    """Build + run on 8 NeuronCores; returns (output, BassKernelResults)."""
    kpb, in_maps = _prepare(features, begins, ends)
    nc = _build_program(kpb)
    res = run_bass_kernel_spmd(nc, in_maps, list(range(B)), trace=trace)
    # out is [P, NBLK, D] fp16 with window w = i*128 + p at [p, i, :]
    out = np.stack(
        [
            np.ascontiguousarray(
                res.results[c]["out"].astype(np.float32).transpose(1, 0, 2)
            ).reshape(W, D)
            for c in range(B)
        ],
        axis=0,
    )
    return out, res


def kernel(features, begins, ends):
    out, _ = run(features, begins, ends, trace=False)
    return out


# revision 21
# speedup vs baseline: 1.1800x; 1.1681x over previous
"""Trainium2 Bass kernel for windowed mean-pooling (segment_reduce).

Computes, for each (batch b, window w):
    out[b, w, :] = mean over t in [begins[b,w], ends'[b,w]) of features[b, t, :]
where ends' = clip(ends, begins, begins + 8) (the reference gathers at most
MAX_WINDOW=8 tokens) and empty windows produce 0 (count clamped to >= 1).

Strategy (data-parallel over batch, one sample per NeuronCore). The kernel is
bound by DMA queue time (each descriptor line costs ~87 ns + bytes/43 GB/s on
one of 16 queues) with the TensorEngine a close second, so the design
minimizes both:
  - TOKEN COMPACTION: for each block of 128 consecutive (sorted) windows,
    the host packs just the distinct tokens that block touches (~230) into
    ceil(n/128) K-tiles -> 39 K-tile matmuls total instead of 62 dense ones
    (PE: ~13 us instead of ~20 us). The K-tile count per block is the max
    over the 8 cores so one SPMD program serves all (padded slots carry
    zero features and an out-of-range token id -> zero mask).
  - features fp16 [P, NTILES, D] in chunks of 2-8 K-tiles -> 3-12 KB
    per-partition DMA lines, split across both HWDGE rings in K order.
  - masks: tiles for the first HBLK blocks ship as a small host-built fp8
    head (the PE consumes fp8 stationary directly vs fp16 moving), so the
    first matmuls skip the on-device broadcast/compare chain; remaining
    tiles are built on DVE from an 8 KB begins/ends row broadcast across
    partitions by K=1 ones-matmuls ((b<=t)*(t<e) vs the shipped token-id
    column of each compacted K-tile).
  - out_block = S^T @ F accumulated in PSUM (512+256 col split), scaled by
    1/count on ACT, written fp16 to [P, NBLK, D] DRAM in groups (8,4,2,1,1)
    (12 KB lines early, small at the latency-critical tail); the host
    un-shuffles and upcasts.
  - warm-up matmuls at t=0 ramp the PE p-state (0.65 -> 2.4 GHz needs ~3 us
    of continuous execution) while the DMA rings spin up (~3 us).
"""

import os
import sys

import numpy as np

for _p in ("/opt/trn_rl_repo", "/root/.axon_site/_ro/trn_rl_repo"):
    if os.path.isdir(_p) and _p not in sys.path:
        sys.path.insert(0, _p)

import ml_dtypes  # noqa: E402

from concourse import bacc, mybir  # noqa: E402
import concourse.tile as tile  # noqa: E402
from concourse.bass_utils import run_bass_kernel_spmd  # noqa: E402

B, T, D, W = 8, 4096, 768, 2048
MAXWIN = 8
P = 128
NBLK = W // P  # 16 window blocks of 128 windows
HBLK = 8  # leading blocks whose masks ship as the host fp8 head
NWARM = 7  # PE warm-up matmuls ([P, 512] each)
MCH = 512  # windows per metadata broadcast matmul
F32 = mybir.dt.float32
FP16 = mybir.dt.float16
FP8 = mybir.dt.float8e4

FP8NP = ml_dtypes.float8_e4m3


def _fchunks(ntiles):
    """Feature DMA chunk sizes (K-tiles): small first for an early PE start,
    12 KB lines later."""
    sizes = [2, 2, 4, 8]
    left = ntiles - sum(sizes)
    while left > 8:
        sizes.append(8)
        left -= 8
    if left:
        sizes.append(left)
    return sizes


def _build_program(kpb):
    """Build the SPMD Bass program given K-tiles-per-block kpb[NBLK]."""
    nc = bacc.Bacc(None)
    k0s = np.concatenate([[0], np.cumsum(kpb)]).astype(int)  # block k offsets
    ntiles = int(k0s[-1])
    assert ntiles + NBLK <= 64, ntiles  # tokid + iv fit the [P, 64] ioiv
    hw8 = int(k0s[HBLK]) * P  # host mask head columns

    fhi_d = nc.declare_dram_parameter("fhi", [P, ntiles, D], FP16, isOutput=False)
    m8_d = nc.declare_dram_parameter("mask8h", [P, hw8], FP8, isOutput=False)
    meta_d = nc.declare_dram_parameter("meta", [1, 2, W], FP16, isOutput=False)
    ioiv_d = nc.declare_dram_parameter("ioiv", [P, 64], F32, isOutput=False)
    out_d = nc.declare_dram_parameter("out", [P, NBLK, D], FP16, isOutput=True)

    fhi_r = fhi_d[:]
    out_r = out_d[:]

    with tile.TileContext(nc) as tc:
        with (
            tc.tile_pool(name="warmp", bufs=1) as warm_pool,
            tc.tile_pool(name="metap", bufs=1) as meta_pool,
            tc.tile_pool(name="fslab", bufs=1) as f_pool,
            tc.tile_pool(name="m2p", bufs=4) as m2_pool,
            tc.tile_pool(name="maskp", bufs=12) as mask_pool,
            tc.tile_pool(name="outp", bufs=3) as out_pool,
            tc.tile_pool(name="psum", bufs=4, space="PSUM") as psum_pool,
        ):
            # --- PE warm-up: ramp the tensor engine p-state while DMAs start.
            warm_sb = warm_pool.tile([P, 512], FP16)
            nc.vector.memset(warm_sb[:], 0.0)
            for j in range(NWARM):
                wp = psum_pool.tile([P, 512], F32, name=f"warm{j}", tag="ps")
                nc.tensor.matmul(
                    wp[:], warm_sb[:, 0:P], warm_sb[:], start=True, stop=True
                )

            # --- metadata DMAs (small, first on the SP ring).
            m8_sb = meta_pool.tile([P, hw8], FP8)
            nc.sync.dma_start(out=m8_sb[:], in_=m8_d[:])
            meta_sb = meta_pool.tile([1, 2, W], FP16)
            nc.sync.dma_start(out=meta_sb[:], in_=meta_d[:])
            ioiv_sb = meta_pool.tile([P, 64], F32)
            nc.sync.dma_start(out=ioiv_sb[:], in_=ioiv_d[:])
            io_sb = ioiv_sb[:, 0:ntiles]
            iv_sb = ioiv_sb[:, ntiles : ntiles + NBLK]

            # --- feature slab chunks (fp16), ALL on the ACT ring: the DMA
            # queues drain lines in enqueue order, so a single ring keeps
            # K-tile arrival in consumption order (two rings interleave by
            # dispatch time and let late-K chunks jump early-K ones).
            fhi_tiles = []
            k2chunk = []
            k0 = 0
            for j, sz in enumerate(_fchunks(ntiles)):
                fh = f_pool.tile([P, sz, D], FP16, name=f"fh{j}", tag=f"fh{j}")
                eng = nc.scalar
                eng.dma_start(out=fh[:], in_=fhi_r[:, k0 : k0 + sz, :])
                fhi_tiles.append(fh)
                for s in range(sz):
                    k2chunk.append((j, s))
                k0 += sz
            assert k0 == ntiles

            # --- broadcast begins/ends across partitions (only the window
            # chunks that device-built mask tiles consume).
            smin = HBLK * P // MCH
            ones_sb = meta_pool.tile([1, P], FP16)
            nc.vector.memset(ones_sb[:], 1.0)
            be_sb = meta_pool.tile([P, 2, W], FP16)
            for s in range(smin, W // MCH):
                for h in range(2):
                    sl = slice(s * MCH, (s + 1) * MCH)
                    pb = psum_pool.tile([P, MCH], F32, name=f"pb{h}_{s}", tag="ps")
                    nc.tensor.matmul(
                        pb[:], ones_sb[:], meta_sb[:, h, sl], start=True, stop=True
                    )
                    nc.vector.tensor_copy(out=be_sb[:, h, sl], in_=pb[:])

            # --- mask tiles for blocks >= HBLK on DVE:
            # mask[p, w] = (b[w] <= tokid) * (tokid < e[w]).
            dmasks = {}
            for i in range(HBLK, NBLK):
                wlo, whi = i * P, (i + 1) * P
                for k in range(kpb[i]):
                    kc = int(k0s[i]) + k
                    m2 = m2_pool.tile([P, P], FP16, name=f"m2_{kc}", tag="m2")
                    msk = mask_pool.tile([P, P], FP16, name=f"mask_{kc}", tag="mask")
                    nc.vector.tensor_scalar(
                        m2[:], be_sb[:, 1, wlo:whi], io_sb[:, kc : kc + 1], None,
                        mybir.AluOpType.is_gt,
                    )
                    nc.vector.scalar_tensor_tensor(
                        msk[:], be_sb[:, 0, wlo:whi], io_sb[:, kc : kc + 1], m2[:],
                        mybir.AluOpType.is_le, mybir.AluOpType.mult,
                    )
                    dmasks[kc] = msk

            # --- block matmuls + ACT evacuation + grouped output DMA.
            ogroups = (8, 4, 4)
            og_starts = []
            o0 = 0
            for g in ogroups:
                og_starts.append(o0)
                o0 += g
            assert o0 == NBLK

            gi = 0
            os_tile = None
            for i in range(NBLK):
                if i == og_starts[gi]:
                    os_tile = out_pool.tile(
                        [P, ogroups[gi], D], FP16, name=f"os{gi}", tag="os"
                    )
                ps = psum_pool.tile([P, D], F32, name=f"ps{i}", tag="ps")
                for k in range(kpb[i]):
                    kc = int(k0s[i]) + k
                    if i < HBLK:
                        lh = m8_sb[:, kc * P : (kc + 1) * P]
                    else:
                        lh = dmasks[kc]
                    cj, cs = k2chunk[kc]
                    rh = fhi_tiles[cj][:, cs, :]
                    first = k == 0
                    last = k == kpb[i] - 1
                    for n0, nn in ((0, 512), (512, 256)):
                        nc.tensor.matmul(
                            ps[:, n0 : n0 + nn], lh, rh[:, n0 : n0 + nn],
                            start=first, stop=(last and n0 == 512),
                        )
                if i % 4 != 3:
                    nc.scalar.mul(
                        out=os_tile[:, i - og_starts[gi], :], in_=ps[:],
                        mul=iv_sb[:, i : i + 1],
                    )
                else:
                    nc.vector.tensor_scalar(
                        os_tile[:, i - og_starts[gi], :], ps[:],
                        iv_sb[:, i : i + 1], None, mybir.AluOpType.mult,
                    )
                if i == og_starts[gi] + ogroups[gi] - 1:
                    # outputs on the SP ring (idle after the metadata).
                    nc.sync.dma_start(
                        out=out_r[:, og_starts[gi] : i + 1, :], in_=os_tile[:]
                    )
                    gi += 1

    nc.finalize()
    return nc


def _prepare(features, begins, ends):
    feats = np.asarray(features, dtype=np.float32)
    assert feats.shape == (B, T, D), feats.shape
    b = np.clip(np.asarray(begins).astype(np.int64), 0, T - 1)
    e = np.asarray(ends).astype(np.int64)
    # Reference gathers at most MAXWIN tokens starting at b; empty -> count 1.
    e_eff = np.clip(e, b, np.minimum(b + MAXWIN, T))
    counts = np.maximum(e_eff - b, 1).astype(np.float32)
    inv = (1.0 / counts).astype(np.float32)

    # distinct tokens per (core, block); K-tiles per block = max over cores.
    toks = {}
    kpb = np.zeros(NBLK, int)
    for c in range(B):
        for i in range(NBLK):
            ws = slice(i * P, (i + 1) * P)
            m = np.zeros(T, bool)
            for bb, ee in zip(b[c, ws], e_eff[c, ws]):
                m[bb:ee] = True
            u = np.flatnonzero(m)
            toks[(c, i)] = u
            kpb[i] = max(kpb[i], (len(u) + P - 1) // P)
    k0s = np.concatenate([[0], np.cumsum(kpb)]).astype(int)
    ntiles = int(k0s[-1])
    hw8 = int(k0s[HBLK]) * P

    f16 = feats.astype(np.float16)
    fhi = np.zeros((B, P, ntiles, D), np.float16)
    tokid = np.full((B, P, ntiles), -3000.0, np.float32)  # pad -> mask 0
    mask8 = np.zeros((B, P, hw8), dtype=FP8NP)
    for c in range(B):
        for i in range(NBLK):
            u = toks[(c, i)]
            n = len(u)
            nk = (n + P - 1) // P
            base = int(k0s[i])
            for k in range(nk):
                seg = u[k * P : (k + 1) * P]
                fhi[c, : len(seg), base + k, :] = f16[c, seg, :]
                tokid[c, : len(seg), base + k] = seg - 2048
            if i < HBLK:
                # host fp8 mask tiles for this block (0/1 exact in fp8)
                wlo = i * P
                t_col = tokid[c, :, base : base + kpb[i]]  # [P, kpb]
                bb = b[c, wlo : wlo + P] - 2048
                ee = e_eff[c, wlo : wlo + P] - 2048
                m = (bb[None, None, :] <= t_col[:, :, None]) & (
                    t_col[:, :, None] < ee[None, None, :]
                )  # [P, kpb, 128w]
                for k in range(kpb[i]):
                    mask8[c, :, (base + k) * P : (base + k + 1) * P] = m[
                        :, k, :
                    ].astype(FP8NP)

    in_maps = []
    for c in range(B):
        metac = np.ascontiguousarray(
            (np.stack([b[c], e_eff[c]]) - 2048).astype(np.float16).reshape(1, 2, W)
        )
        ioiv = np.zeros((P, 64), np.float32)
        ioiv[:, 0:ntiles] = tokid[c]
        ioiv[:, ntiles : ntiles + NBLK] = inv[c].reshape(NBLK, P).T
        in_maps.append(
            {"fhi": fhi[c], "mask8h": mask8[c], "meta": metac, "ioiv": ioiv}
        )
    return list(kpb), in_maps


def run(features, begins, ends, trace=False):
    """Build + run on 8 NeuronCores; returns (output, BassKernelResults)."""
    kpb, in_maps = _prepare(features, begins, ends)
    nc = _build_program(kpb)
    res = run_bass_kernel_spmd(nc, in_maps, list(range(B)), trace=trace)
    # out is [P, NBLK, D] fp16 with window w = i*128 + p at [p, i, :]
    out = np.stack(
        [
            np.ascontiguousarray(
                res.results[c]["out"].astype(np.float32).transpose(1, 0, 2)
            ).reshape(W, D)
            for c in range(B)
        ],
        axis=0,
    )
    return out, res


def kernel(features, begins, ends):
    out, _ = run(features, begins, ends, trace=False)
    return out


# revision 22
# speedup vs baseline: 1.1941x; 1.0120x over previous
"""Trainium2 Bass kernel for windowed mean-pooling (segment_reduce).

Computes, for each (batch b, window w):
    out[b, w, :] = mean over t in [begins[b,w], ends'[b,w]) of features[b, t, :]
where ends' = clip(ends, begins, begins + 8) (the reference gathers at most
MAX_WINDOW=8 tokens) and empty windows produce 0 (count clamped to >= 1).

Strategy (data-parallel over batch, one sample per NeuronCore). The kernel is
bound by DMA queue time (each descriptor line costs ~87 ns + bytes/43 GB/s on
one of 16 queues) with the TensorEngine a close second, so the design
minimizes both:
  - TOKEN COMPACTION: for each block of 128 consecutive (sorted) windows,
    the host packs just the distinct tokens that block touches (~230) into
    ceil(n/128) K-tiles -> 39 K-tile matmuls total instead of 62 dense ones
    (PE: ~13 us instead of ~20 us). The K-tile count per block is the max
    over the 8 cores so one SPMD program serves all (padded slots carry
    zero features and an out-of-range token id -> zero mask).
  - features fp16 [P, NTILES, D] in chunks of 2-8 K-tiles -> 3-12 KB
    per-partition DMA lines, split across both HWDGE rings in K order.
  - masks: tiles for the first HBLK blocks ship as a small host-built fp8
    head (the PE consumes fp8 stationary directly vs fp16 moving), so the
    first matmuls skip the on-device broadcast/compare chain; remaining
    tiles are built on DVE from an 8 KB begins/ends row broadcast across
    partitions by K=1 ones-matmuls ((b<=t)*(t<e) vs the shipped token-id
    column of each compacted K-tile).
  - out_block = S^T @ F accumulated in PSUM (512+256 col split), scaled by
    1/count on ACT, written fp16 to [P, NBLK, D] DRAM in groups (8,4,2,1,1)
    (12 KB lines early, small at the latency-critical tail); the host
    un-shuffles and upcasts.
  - warm-up matmuls at t=0 ramp the PE p-state (0.65 -> 2.4 GHz needs ~3 us
    of continuous execution) while the DMA rings spin up (~3 us).
"""

import os
import sys

import numpy as np

for _p in ("/opt/trn_rl_repo", "/root/.axon_site/_ro/trn_rl_repo"):
    if os.path.isdir(_p) and _p not in sys.path:
        sys.path.insert(0, _p)

import ml_dtypes  # noqa: E402

from concourse import bacc, mybir  # noqa: E402
import concourse.tile as tile  # noqa: E402
from concourse.bass_utils import run_bass_kernel_spmd  # noqa: E402

B, T, D, W = 8, 4096, 768, 2048
MAXWIN = 8
P = 128
NBLK = W // P  # 16 window blocks of 128 windows
HBLK = 10  # leading blocks whose masks ship as the host fp8 head
NWARM = 7  # PE warm-up matmuls ([P, 512] each)
MCH = 512  # windows per metadata broadcast matmul
F32 = mybir.dt.float32
FP16 = mybir.dt.float16
FP8 = mybir.dt.float8e4

FP8NP = ml_dtypes.float8_e4m3


def _fchunks(ntiles):
    """Feature DMA chunk sizes (K-tiles): small first for an early PE start,
    12 KB lines later."""
    sizes = [2, 2, 4, 4, 4]
    left = ntiles - sum(sizes)
    while left > 8:
        sizes.append(8)
        left -= 8
    if left:
        sizes.append(left)
    return sizes


def _build_program(kpb):
    """Build the SPMD Bass program given K-tiles-per-block kpb[NBLK]."""
    nc = bacc.Bacc(None)
    k0s = np.concatenate([[0], np.cumsum(kpb)]).astype(int)  # block k offsets
    ntiles = int(k0s[-1])
    assert ntiles + NBLK <= 64, ntiles  # tokid + iv fit the [P, 64] ioiv
    hw8 = int(k0s[HBLK]) * P  # host mask head columns

    fhi_d = nc.declare_dram_parameter("fhi", [P, ntiles, D], FP16, isOutput=False)
    m8_d = nc.declare_dram_parameter("mask8h", [P, hw8], FP8, isOutput=False)
    meta_d = nc.declare_dram_parameter("meta", [1, 2, W], FP16, isOutput=False)
    ioiv_d = nc.declare_dram_parameter("ioiv", [P, 64], F32, isOutput=False)
    out_d = nc.declare_dram_parameter("out", [P, NBLK, D], FP16, isOutput=True)

    fhi_r = fhi_d[:]
    out_r = out_d[:]

    with tile.TileContext(nc) as tc:
        with (
            tc.tile_pool(name="warmp", bufs=1) as warm_pool,
            tc.tile_pool(name="metap", bufs=1) as meta_pool,
            tc.tile_pool(name="fslab", bufs=1) as f_pool,
            tc.tile_pool(name="m2p", bufs=4) as m2_pool,
            tc.tile_pool(name="maskp", bufs=12) as mask_pool,
            tc.tile_pool(name="outp", bufs=3) as out_pool,
            tc.tile_pool(name="psum", bufs=4, space="PSUM") as psum_pool,
        ):
            # --- PE warm-up: ramp the tensor engine p-state while DMAs start.
            warm_sb = warm_pool.tile([P, 512], FP16)
            nc.vector.memset(warm_sb[:], 0.0)
            for j in range(NWARM):
                wp = psum_pool.tile([P, 512], F32, name=f"warm{j}", tag="ps")
                nc.tensor.matmul(
                    wp[:], warm_sb[:, 0:P], warm_sb[:], start=True, stop=True
                )

            # --- metadata DMAs (small, first on the SP ring).
            m8_sb = meta_pool.tile([P, hw8], FP8)
            nc.sync.dma_start(out=m8_sb[:], in_=m8_d[:])
            meta_sb = meta_pool.tile([1, 2, W], FP16)
            nc.sync.dma_start(out=meta_sb[:], in_=meta_d[:])
            ioiv_sb = meta_pool.tile([P, 64], F32)
            nc.sync.dma_start(out=ioiv_sb[:], in_=ioiv_d[:])
            io_sb = ioiv_sb[:, 0:ntiles]
            iv_sb = ioiv_sb[:, ntiles : ntiles + NBLK]

            # --- feature slab chunks (fp16), ALL on the ACT ring: the DMA
            # queues drain lines in enqueue order, so a single ring keeps
            # K-tile arrival in consumption order (two rings interleave by
            # dispatch time and let late-K chunks jump early-K ones).
            fhi_tiles = []
            k2chunk = []
            k0 = 0
            for j, sz in enumerate(_fchunks(ntiles)):
                fh = f_pool.tile([P, sz, D], FP16, name=f"fh{j}", tag=f"fh{j}")
                eng = nc.scalar
                eng.dma_start(out=fh[:], in_=fhi_r[:, k0 : k0 + sz, :])
                fhi_tiles.append(fh)
                for s in range(sz):
                    k2chunk.append((j, s))
                k0 += sz
            assert k0 == ntiles

            # --- broadcast begins/ends across partitions (only the window
            # chunks that device-built mask tiles consume).
            smin = HBLK * P // MCH
            ones_sb = meta_pool.tile([1, P], FP16)
            nc.vector.memset(ones_sb[:], 1.0)
            be_sb = meta_pool.tile([P, 2, W], FP16)
            for s in range(smin, W // MCH):
                for h in range(2):
                    sl = slice(s * MCH, (s + 1) * MCH)
                    pb = psum_pool.tile([P, MCH], F32, name=f"pb{h}_{s}", tag="ps")
                    nc.tensor.matmul(
                        pb[:], ones_sb[:], meta_sb[:, h, sl], start=True, stop=True
                    )
                    nc.vector.tensor_copy(out=be_sb[:, h, sl], in_=pb[:])

            # --- mask tiles for blocks >= HBLK on DVE:
            # mask[p, w] = (b[w] <= tokid) * (tokid < e[w]).
            dmasks = {}
            for i in range(HBLK, NBLK):
                wlo, whi = i * P, (i + 1) * P
                for k in range(kpb[i]):
                    kc = int(k0s[i]) + k
                    m2 = m2_pool.tile([P, P], FP16, name=f"m2_{kc}", tag="m2")
                    msk = mask_pool.tile([P, P], FP16, name=f"mask_{kc}", tag="mask")
                    nc.vector.tensor_scalar(
                        m2[:], be_sb[:, 1, wlo:whi], io_sb[:, kc : kc + 1], None,
                        mybir.AluOpType.is_gt,
                    )
                    nc.vector.scalar_tensor_tensor(
                        msk[:], be_sb[:, 0, wlo:whi], io_sb[:, kc : kc + 1], m2[:],
                        mybir.AluOpType.is_le, mybir.AluOpType.mult,
                    )
                    dmasks[kc] = msk

            # --- block matmuls + ACT evacuation + grouped output DMA.
            ogroups = (8, 4, 4)
            og_starts = []
            o0 = 0
            for g in ogroups:
                og_starts.append(o0)
                o0 += g
            assert o0 == NBLK

            gi = 0
            os_tile = None
            for i in range(NBLK):
                if i == og_starts[gi]:
                    os_tile = out_pool.tile(
                        [P, ogroups[gi], D], FP16, name=f"os{gi}", tag="os"
                    )
                ps = psum_pool.tile([P, D], F32, name=f"ps{i}", tag="ps")
                for k in range(kpb[i]):
                    kc = int(k0s[i]) + k
                    if i < HBLK:
                        lh = m8_sb[:, kc * P : (kc + 1) * P]
                    else:
                        lh = dmasks[kc]
                    cj, cs = k2chunk[kc]
                    rh = fhi_tiles[cj][:, cs, :]
                    first = k == 0
                    last = k == kpb[i] - 1
                    for n0, nn in ((0, 512), (512, 256)):
                        nc.tensor.matmul(
                            ps[:, n0 : n0 + nn], lh, rh[:, n0 : n0 + nn],
                            start=first, stop=(last and n0 == 512),
                        )
                if i % 4 != 3:
                    nc.scalar.mul(
                        out=os_tile[:, i - og_starts[gi], :], in_=ps[:],
                        mul=iv_sb[:, i : i + 1],
                    )
                else:
                    nc.vector.tensor_scalar(
                        os_tile[:, i - og_starts[gi], :], ps[:],
                        iv_sb[:, i : i + 1], None, mybir.AluOpType.mult,
                    )
                if i == og_starts[gi] + ogroups[gi] - 1:
                    # outputs on the SP ring (idle after the metadata).
                    nc.sync.dma_start(
                        out=out_r[:, og_starts[gi] : i + 1, :], in_=os_tile[:]
                    )
                    gi += 1

    nc.finalize()
    return nc


def _prepare(features, begins, ends):
    feats = np.asarray(features, dtype=np.float32)
    assert feats.shape == (B, T, D), feats.shape
    b = np.clip(np.asarray(begins).astype(np.int64), 0, T - 1)
    e = np.asarray(ends).astype(np.int64)
    # Reference gathers at most MAXWIN tokens starting at b; empty -> count 1.
    e_eff = np.clip(e, b, np.minimum(b + MAXWIN, T))
    counts = np.maximum(e_eff - b, 1).astype(np.float32)
    inv = (1.0 / counts).astype(np.float32)

    # distinct tokens per (core, block); K-tiles per block = max over cores.
    toks = {}
    kpb = np.zeros(NBLK, int)
    for c in range(B):
        for i in range(NBLK):
            ws = slice(i * P, (i + 1) * P)
            m = np.zeros(T, bool)
            for bb, ee in zip(b[c, ws], e_eff[c, ws]):
                m[bb:ee] = True
            u = np.flatnonzero(m)
            toks[(c, i)] = u
            kpb[i] = max(kpb[i], (len(u) + P - 1) // P)
    k0s = np.concatenate([[0], np.cumsum(kpb)]).astype(int)
    ntiles = int(k0s[-1])
    hw8 = int(k0s[HBLK]) * P

    f16 = feats.astype(np.float16)
    fhi = np.zeros((B, P, ntiles, D), np.float16)
    tokid = np.full((B, P, ntiles), -3000.0, np.float32)  # pad -> mask 0
    mask8 = np.zeros((B, P, hw8), dtype=FP8NP)
    for c in range(B):
        for i in range(NBLK):
            u = toks[(c, i)]
            n = len(u)
            nk = (n + P - 1) // P
            base = int(k0s[i])
            for k in range(nk):
                seg = u[k * P : (k + 1) * P]
                fhi[c, : len(seg), base + k, :] = f16[c, seg, :]
                tokid[c, : len(seg), base + k] = seg - 2048
            if i < HBLK:
                # host fp8 mask tiles for this block (0/1 exact in fp8)
                wlo = i * P
                t_col = tokid[c, :, base : base + kpb[i]]  # [P, kpb]
                bb = b[c, wlo : wlo + P] - 2048
                ee = e_eff[c, wlo : wlo + P] - 2048
                m = (bb[None, None, :] <= t_col[:, :, None]) & (
                    t_col[:, :, None] < ee[None, None, :]
                )  # [P, kpb, 128w]
                for k in range(kpb[i]):
                    mask8[c, :, (base + k) * P : (base + k + 1) * P] = m[
                        :, k, :
                    ].astype(FP8NP)

    in_maps = []
    for c in range(B):
        metac = np.ascontiguousarray(
            (np.stack([b[c], e_eff[c]]) - 2048).astype(np.float16).reshape(1, 2, W)
        )
        ioiv = np.zeros((P, 64), np.float32)
        ioiv[:, 0:ntiles] = tokid[c]
        ioiv[:, ntiles : ntiles + NBLK] = inv[c].reshape(NBLK, P).T
        in_maps.append(
            {"fhi": fhi[c], "mask8h": mask8[c], "meta": metac, "ioiv": ioiv}
        )
    return list(kpb), in_maps


def run(features, begins, ends, trace=False):
    """Build + run on 8 NeuronCores; returns (output, BassKernelResults)."""
    kpb, in_maps = _prepare(features, begins, ends)
    nc = _build_program(kpb)
    res = run_bass_kernel_spmd(nc, in_maps, list(range(B)), trace=trace)
    # out is [P, NBLK, D] fp16 with window w = i*128 + p at [p, i, :]
    out = np.stack(
        [
            np.ascontiguousarray(
                res.results[c]["out"].astype(np.float32).transpose(1, 0, 2)
            ).reshape(W, D)
            for c in range(B)
        ],
        axis=0,
    )
    return out, res


def kernel(features, begins, ends):
    out, _ = run(features, begins, ends, trace=False)
    return out
